# revision 2
# baseline (speedup 1.0000x reference)
"""ConvACNVQVAE forward on 8 trn2 NeuronCores, data-parallel over batch.

Strategy (per spec sharding_hint): batch 128 -> 8 shards of 16 images; all
weights (codebook, convs, linears) replicated on every core. Each core runs
the full forward on its shard; results are concatenated on the host. No
cross-core communication is needed for eval-mode forward.

Matmuls/convs are forced to true fp32 (no bf16 autocast) because the VQ
argmin margins require fp32-accurate distances.
"""
import os

# Must be set before the neuron compiler first runs: keep fp32 exact.
_flags = os.environ.get("NEURON_CC_FLAGS", "")
if "--auto-cast" not in _flags:
    os.environ["NEURON_CC_FLAGS"] = (_flags + " --auto-cast=none").strip()

import numpy as np

H = 256      # hidden_size
CODE = 512   # code_len
B = 128
K = 512      # num_embeddings
D = H        # embedding_dim
BN_EPS = 1e-5
N_CORES = 8


def _np(x):
    return np.asarray(x)


def _forward_jax(inputs, params):
    """Identical math to the reference model (eval mode)."""
    import jax
    import jax.numpy as jnp
    from jax import lax

    def _conv(x, p, pad):
        y = lax.conv_general_dilated(x, p['w'], (1, 1), [(pad, pad), (pad, pad)],
                                     dimension_numbers=('NCHW', 'OIHW', 'NCHW'))
        return y + p['b'][None, :, None, None]

    def _convt(x, p, k):
        w = jnp.transpose(jnp.flip(p['w'], (2, 3)), (1, 0, 2, 3))
        y = lax.conv_general_dilated(x, w, (1, 1), [(k - 1, k - 1)] * 2,
                                     dimension_numbers=('NCHW', 'OIHW', 'NCHW'))
        return y + p['b'][None, :, None, None]

    def _bn(x, p):
        scale = (p['g'] / jnp.sqrt(p['v'] + BN_EPS))[None, :, None, None]
        return (x - p['m'][None, :, None, None]) * scale + p['b'][None, :, None, None]

    def _res(x, p):
        y = _bn(_conv(jax.nn.relu(x), p['c1'], 1), p['bn1'])
        y = _bn(_conv(jax.nn.relu(y), p['c2'], 0), p['bn2'])
        return y + x

    def _lin(x, p):
        return x @ p['w'].T + p['b']

    p = params
    h1 = jax.nn.relu(_conv(inputs, p['c1'], 0))
    h2 = jax.nn.relu(_conv(h1, p['c2'], 0))
    h3 = jax.nn.relu(_conv(h2, p['c3'], 0))
    r2 = _res(_res(h3, p['res1']), p['res2'])
    fh = _conv(_conv(r2, p['c4'], 1), p['c5'], 1)
    mu_i = _conv(fh, p['c5_mu'], 0)
    ls_i = _conv(fh, p['c5_log_std'], 0)
    shp = mu_i.shape
    acn_mu_flat = _lin(mu_i.reshape(shp[0], -1), p['l5_mu'])
    acn_log_std_flat = _lin(ls_i.reshape(shp[0], -1), p['l5_log_std'])
    acn_z = _lin(acn_mu_flat, p['il5']).reshape(shp)
    vq_e_z = _conv(_res(jax.nn.relu(_bn(_conv(acn_z, p['c1_v'], 0), p['c1_v_bn'])),
                        p['res1_v']), p['c1_v_last'], 0)

    x = jnp.transpose(vq_e_z, (0, 2, 3, 1))
    flat = x.reshape(-1, D)
    E = p['embedding']
    dist = (jnp.sum(flat * flat, axis=1, keepdims=True)
            + jnp.sum(E * E, axis=1) - 2.0 * flat @ E.T)
    idx = jnp.argmin(dist, axis=1)
    quant = jnp.take(E, idx, axis=0).reshape(x.shape)
    enc = jax.nn.one_hot(idx, K, dtype=x.dtype)
    vq_indices = jnp.transpose(enc.reshape(-1, 16, 16, K), (0, 3, 1, 2))
    vq_q_z = jnp.transpose(quant, (0, 3, 1, 2))

    ih1 = _convt(vq_q_z, p['ic4'], 1)
    ir2 = _res(_res(ih1, p['ires1']), p['ires2'])
    ih2 = jax.nn.relu(_convt(ir2, p['ic3'], 5))
    ih3 = jax.nn.relu(_convt(ih2, p['ic2'], 5))
    out = _convt(_convt(ih3, p['ic1'], 5), p['ic1_o'], 1)
    return (out, acn_mu_flat, acn_log_std_flat, vq_e_z, vq_q_z, vq_indices)


def _run_pmap(inputs, params):
    """Data-parallel over 8 NeuronCores via pmap; weights replicated."""
    import jax

    try:
        devs = jax.devices('axon')
    except Exception:
        devs = jax.devices()
    n = min(N_CORES, len(devs))
    assert inputs.shape[0] % n == 0, (inputs.shape, n)
    sh = inputs.reshape(n, inputs.shape[0] // n, *inputs.shape[1:])
    fwd = jax.pmap(_forward_jax, in_axes=(0, None), devices=devs[:n])
    outs = fwd(sh, params)
    return tuple(
        _np(o).reshape(-1, *o.shape[2:]) for o in outs
    )


def kernel(inputs, params):
    inputs = np.asarray(inputs, dtype=np.float32)
    params = __import__("jax").tree_util.tree_map(
        lambda a: np.asarray(a, dtype=np.float32), params)
    try:
        outs = _run_pmap(inputs, params)
    except Exception:
        # Fallback: single-device (CPU or default) execution, still correct.
        outs = tuple(_np(o) for o in _forward_jax(inputs, params))
    return outs
